# revision 21
# baseline (speedup 1.0000x reference)
"""Trainium2 Bass kernel for nn_Autograd4bitQuantLinear (4-bit quant linear).

Computes out = x @ dequant4(qweight, scales, zeros) + bias where
  x:       (4, 2048, 4096) f32
  qweight: (512, 11008)    i32  (8 nibbles packed per int32 along rows)
  scales:  (11008, 1)      f32
  zeros:   (11008, 1)      f32
  bias:    (11008,)        f32
  out:     (4, 2048, 11008) f32

Strategy (tensor-parallel over 8 NeuronCores, column-sharded out_features):
  - Each core owns 1376 output columns; x is replicated.
  - On-device dequant: nibble-unpack qweight int32 (DVE shift/and with
    per-partition shift amounts), fold scale/zero (W = q * s - z), W bf16.
  - W is NOT kept SBUF-resident: it is produced just-in-time for chunk 0
    (DVE unpack at ~1.6us/k-tile vs PE consumption 1.73us/k-tile), spilled
    to a DRAM scratch, and re-streamed per chunk afterwards. This frees
    ~65KB/partition of SBUF for the x-tile pool.
  - x is cast f32->bf16 by SWDGE cast-DMAs into DRAM scratch (chunk 0 in
    8 column slices so the first transpose lands ~8us in), then
    DMA-transposed (xbar) into SBUF as [128, 1024] k-major tiles. The
    68-tile xt pool double-buffers whole chunks, so each chunk's 32
    transposes run entirely during the previous chunk's compute (the
    transpose issue+wait cost of ~2.6us each made transpose supply the
    bottleneck of earlier versions).
  - PE: chunks of 1024 rows, group-major k-outer with mt-inner over all
    8 PSUM banks: out[m, n] accumulated over 32 k-tiles (bf16 -> f32).
  - Queue roles: sync = transposes; scalar = output stores; gpsimd
    (SWDGE) = casts, qweight loads, W spills/loads, broadcasts. (Issuing
    transposes on the scalar queue corrupts data on HW - keep them on
    sync only.)
  - Epilogue per (group, mt): psum + bias (DVE) -> SBUF -> store.
"""

import sys

sys.path.insert(0, "/opt/trn_rl_repo")

import numpy as np

import concourse.bass as bass
import concourse.mybir as mybir
from concourse import bacc
from concourse.tile import TileContext
from concourse.tile_rust import add_dep_helper


dt = mybir.dt
AL = mybir.AluOpType

P = 128
IN = 4096  # contraction dim (in_features)
OUT = 11008  # out_features
M_ROWS = 8192  # 4 * 2048
NCORES = 8
NSH = OUT // NCORES  # 1376 output columns per core
KT = IN // P  # 32 k-tiles
MC = 1024  # rows per chunk
# n-chunks within the per-core shard; each must fit one PSUM bank (<=512 f32)
N_CHUNKS = ((0, 512), (512, 512), (1024, 352))
XT_BUFS = 68  # [128, 1024] bf16 tiles; two full chunks + slack
W_BUFS = 8  # streaming W tiles in flight per group


def build(m_rows=M_ROWS, debug=False):
    """Build + compile the single-core Tile program (SPMD: same on all cores)."""
    assert m_rows % MC == 0
    nc = bacc.Bacc(None, target_bir_lowering=False, debug=debug)

    x_d = nc.dram_tensor("x", [m_rows, IN], dt.float32, kind="ExternalInput")
    qw_d = nc.dram_tensor("qw", [IN, NSH], dt.int32, kind="ExternalInput")
    s_d = nc.dram_tensor("scales", [NSH], dt.float32, kind="ExternalInput")
    z_d = nc.dram_tensor("zeros", [NSH], dt.float32, kind="ExternalInput")
    b_d = nc.dram_tensor("bias", [NSH], dt.float32, kind="ExternalInput")
    shamt_d = nc.dram_tensor("shamt", [P, 1], dt.int32, kind="ExternalInput")
    out_d = nc.dram_tensor("out", [m_rows, NSH], dt.float32, kind="ExternalOutput")

    n_chunks = m_rows // MC
    mt_per_chunk = MC // P

    with TileContext(nc) as tc:
        with (
            tc.tile_pool(name="singles", bufs=1) as singles,
            tc.tile_pool(name="w", bufs=W_BUFS) as wpool,
            tc.tile_pool(name="unpack", bufs=4) as upool,
            tc.tile_pool(name="xbf0", bufs=8, space="DRAM") as xbf0pool,
            tc.tile_pool(name="xbf", bufs=2, space="DRAM") as xbfpool,
            tc.tile_pool(name="wdr", bufs=1, space="DRAM") as wdrpool,
            tc.tile_pool(name="xt", bufs=XT_BUFS) as xtpool,
            tc.tile_pool(name="osb", bufs=2) as opool,
            tc.tile_pool(name="ps", bufs=1, space="PSUM") as pspool,
        ):
            # ---- constants ----
            s_rep = singles.tile([P, NSH], dt.float32, tag="s_rep")
            nc.gpsimd.dma_start(out=s_rep[:], in_=s_d[None, :].to_broadcast([P, NSH]))
            z_rep = singles.tile([P, NSH], dt.float32, tag="z_rep")
            nc.gpsimd.dma_start(out=z_rep[:], in_=z_d[None, :].to_broadcast([P, NSH]))
            shamt = singles.tile([P, 1], dt.int32, tag="shamt")
            nc.scalar.dma_start(out=shamt[:], in_=shamt_d[:])
            mask = singles.tile([P, 1], dt.int32, tag="mask")
            nc.vector.memset(mask[:], 15)

            # ---- chunk-0 x staging: 8 column-slice casts ----
            xbf0 = []
            for j in range(8):
                t = xbf0pool.tile([MC, 512], dt.bfloat16, tag=f"xbf0_{j}",
                                  name=f"xbf0_{j}")
                nc.gpsimd.dma_start(out=t[:], in_=x_d[0:MC, j * 512 : (j + 1) * 512])
                xbf0.append(t)

            b_rep = singles.tile([P, NSH], dt.float32, tag="b_rep")
            nc.gpsimd.dma_start(out=b_rep[:], in_=b_d[None, :].to_broadcast([P, NSH]))

            wdram = wdrpool.tile([IN, NSH], dt.bfloat16, tag="wdram", name="wdram")

            last_xpose = {}
            xbf_steady = {}

            def cast_chunk(c):
                r0 = c * MC
                t = xbfpool.tile([MC, IN], dt.bfloat16, tag="xbf", name=f"xbf{c}")
                ci = nc.gpsimd.dma_start(out=t[:], in_=x_d[r0 : r0 + MC, :])
                if c - 2 in last_xpose:
                    add_dep_helper(
                        ci.ins,
                        last_xpose[c - 2].ins,
                        sync=True,
                        reason="throttle x cast chain",
                    )
                xbf_steady[c] = t

            def transpose_chunk(c):
                """32 xbar transposes -> [128, 1024] tiles on the sync queue."""
                xts = []
                for k in range(KT):
                    xt = xtpool.tile([P, MC], dt.bfloat16, tag="xt", name="xt")
                    if c == 0:
                        j = k // 4
                        src = xbf0[j][:, k * 128 - j * 512 : (k + 1) * 128 - j * 512]
                    else:
                        src = xbf_steady[c][:, k * P : (k + 1) * P]
                    ti = nc.sync.dma_start(out=xt[:], in_=src, transpose=True)
                    xts.append(xt)
                last_xpose[c] = ti
                return xts

            # ---- W dequant (chunk-0 JIT) + spill to DRAM ----
            def unpack_group(i):
                """Returns wtiles for chunk 0's group-i pass; spills to wdram."""
                o, wd = N_CHUNKS[i]
                qts = []
                for k in range(KT):
                    qt = upool.tile([P, wd], dt.int32, tag="qt", name="qt")
                    nc.gpsimd.dma_start(
                        out=qt[:], in_=qw_d[k * P : (k + 1) * P, o : o + wd]
                    )
                    qts.append(qt)
                wtiles = {}
                for k in range(KT):
                    nib = upool.tile([P, wd], dt.int32, tag="nib", name="nib",
                                     bufs=1)
                    nc.vector.scalar_tensor_tensor(
                        nib[:],
                        qts[k][:],
                        shamt[:, 0:1],
                        mask[:, 0:1].to_broadcast([P, wd]),
                        AL.logical_shift_right,
                        AL.bitwise_and,
                    )
                    ws = upool.tile([P, wd], dt.float32, tag="ws", name="ws",
                                    bufs=1)
                    nc.vector.tensor_tensor(
                        ws[:], nib[:], s_rep[:, o : o + wd], AL.mult
                    )
                    wt = wpool.tile([P, wd], dt.bfloat16, tag=f"w{i}",
                                    name=f"w{i}_{k}")
                    nc.vector.tensor_tensor(
                        wt[:], ws[:], z_rep[:, o : o + wd], AL.subtract
                    )
                    nc.gpsimd.dma_start(
                        out=wdram[k * P : (k + 1) * P, o : o + wd], in_=wt[:]
                    )
                    wtiles[k] = wt
                return wtiles

            W_HEAD = 8  # == W_BUFS: head prefetches a full rotation's worth

            def wload_head(i):
                """Prefetch the first W_HEAD W tiles of group i on the scalar
                queue (issued a group or chunk ahead; their pool buffers are
                free by then, so they never block the queue)."""
                o, wd = N_CHUNKS[i]
                wtiles = {}
                for k in range(W_HEAD):
                    wt = wpool.tile([P, wd], dt.bfloat16, tag=f"w{i}",
                                    name=f"w{i}_{k}")
                    nc.scalar.dma_start(
                        out=wt[:], in_=wdram[k * P : (k + 1) * P, o : o + wd]
                    )
                    wtiles[k] = wt
                return wtiles

            def wload_tail(i, wtiles):
                """JIT-stream the remaining W tiles of group i on gpsimd;
                paced by pool rotation against the PE's k-consumption."""
                o, wd = N_CHUNKS[i]
                for k in range(W_HEAD, KT):
                    wt = wpool.tile([P, wd], dt.bfloat16, tag=f"w{i}",
                                    name=f"w{i}_{k}")
                    nc.gpsimd.dma_start(
                        out=wt[:], in_=wdram[k * P : (k + 1) * P, o : o + wd]
                    )
                    wtiles[k] = wt
                return wtiles

            def chunk_group(c, i, xts, wtiles):
                o, wd = N_CHUNKS[i]
                pss = [
                    pspool.tile([P, wd], dt.float32, tag=f"ps{m}", name=f"ps{m}")
                    for m in range(mt_per_chunk)
                ]
                for k in range(KT):
                    for mt in range(mt_per_chunk):
                        nc.tensor.matmul(
                            pss[mt][:],
                            xts[k][:, mt * P : (mt + 1) * P],
                            wtiles[k][:],
                            start=(k == 0),
                            stop=(k == KT - 1),
                        )
                for mt in range(mt_per_chunk):
                    ob = opool.tile([P, wd], dt.float32, tag=f"ob{i}",
                                    name=f"ob{i}")
                    nc.vector.tensor_tensor(
                        ob[:], pss[mt][:], b_rep[:, o : o + wd], AL.add
                    )
                    row = c * MC + mt * P
                    nc.scalar.dma_start(
                        out=out_d[row : row + P, o : o + wd], in_=ob[:]
                    )

            # ---- program ----
            xts_cur = transpose_chunk(0)
            if n_chunks > 1:
                cast_chunk(1)
            xts_next = None
            head0 = None
            for i in range(3):
                wt0 = unpack_group(i)
                chunk_group(0, i, xts_cur, wt0)
                if i == 1:
                    if n_chunks > 1:
                        xts_next = transpose_chunk(1)
                    if n_chunks > 2:
                        cast_chunk(2)
                    if n_chunks > 1:
                        head0 = wload_head(0)

            for c in range(1, n_chunks):
                xts = xts_next
                wts0 = wload_tail(0, head0)
                head1 = wload_head(1)
                chunk_group(c, 0, xts, wts0)
                wts1 = wload_tail(1, head1)
                head2 = wload_head(2)
                chunk_group(c, 1, xts, wts1)
                if c + 1 < n_chunks:
                    xts_next = transpose_chunk(c + 1)
                if c + 2 < n_chunks:
                    cast_chunk(c + 2)
                if c + 1 < n_chunks:
                    head0 = wload_head(0)
                wts2 = wload_tail(2, head2)
                chunk_group(c, 2, xts, wts2)

    nc.compile()
    return nc


_SHAMT = (4 * (np.arange(P, dtype=np.int32) % 8)).reshape(P, 1)


def make_in_maps(x2d, qweight, scales, zeros, bias):
    """Per-core input maps (host-side sharding / layout prep only)."""
    in_maps = []
    for c in range(NCORES):
        sl = slice(c * NSH, (c + 1) * NSH)
        in_maps.append(
            {
                "x": x2d,
                "qw": np.ascontiguousarray(
                    np.repeat(qweight[:, sl], 8, axis=0)
                ),
                "scales": np.ascontiguousarray(scales[sl, 0]),
                "zeros": np.ascontiguousarray(zeros[sl, 0]),
                "bias": np.ascontiguousarray(bias[sl]),
                "shamt": _SHAMT,
            }
        )
    return in_maps


_NC_CACHE = {}


def _get_nc(m_rows):
    if m_rows not in _NC_CACHE:
        _NC_CACHE[m_rows] = build(m_rows)
    return _NC_CACHE[m_rows]


def run_spmd(x2d, qweight, scales, zeros, bias, trace=False, **kwargs):
    """Run on the 8 NeuronCores; returns (out2d [8192, 11008] f32, results)."""
    from concourse.bass_utils import run_bass_kernel_spmd

    m_rows = x2d.shape[0]
    nc = _get_nc(m_rows)
    in_maps = make_in_maps(x2d, qweight, scales, zeros, bias)
    res = run_bass_kernel_spmd(
        nc, in_maps, list(range(NCORES)), trace=trace, **kwargs
    )
    outs = [res.results[c]["out"] for c in range(NCORES)]
    out2d = np.concatenate(outs, axis=1)
    return out2d, res


def kernel(x, qweight, scales, zeros, bias):
    x = np.asarray(x, dtype=np.float32)
    qweight = np.asarray(qweight, dtype=np.int32)
    scales = np.asarray(scales, dtype=np.float32)
    zeros = np.asarray(zeros, dtype=np.float32)
    bias = np.asarray(bias, dtype=np.float32)

    b, s, k_in = x.shape
    x2d = np.ascontiguousarray(x.reshape(b * s, k_in))
    out2d, _ = run_spmd(x2d, qweight, scales, zeros, bias)
    return out2d.reshape(b, s, OUT)
